# revision 71
# baseline (speedup 1.0000x reference)
"""DualBranchMoENet on Trainium2 — 8-core data-parallel (16 samples/core).

Key design points (TimelineSim 251484 ns vs 845489 ns baseline):
- DMA-count collapse: packed const blobs, host-side im2col for the line
  conv1 ([14,8,L] tap-stacked DRAM tensor), expert weights host-relaid
  to SBUF layout, zero-pads via const*0 DVE ops instead of DMAs. Every
  DMA costs ~625ns on the single HWDGE queue, so count dominates.
- f32r matmuls everywhere (1 cyc/row at free-dim>=256 vs 4 for fp32).
- Fully transposed LSTM: gates live on partitions [128gates x 16batch];
  16 tiny whh matmuls/step in bf16 (1 cyc/row), the x-transform is
  pre-accumulated into the gate PSUM by an identity matmul, gate order
  host-permuted to i,f,o,g with g-rows scaled 2x so ONE sigmoid covers
  all gates (tanh x = 2*sig(2x)-1); no per-step transposes or DMAs.
- X-transform precompute split: accF half hidden under conv2; the li
  k-chunks of t<16 finish right after the interp (Act-copy, no DVE
  adds) so the LSTM starts immediately; t>=16 lands during early steps.
- Engine balancing: SE sums via Act accum_out, expert weighted-sum
  split across DVE+GpSimd (note: GpSimd TensorTensor supports mult/add
  but NOT max, and cannot do free-axis reduces or touch PSUM).
- PSUM 2/6 bank split (psA/psB); walrus caps 1 sync-wait per TPB inst
  (extras hoisted onto NoOps by _split_tpb_waits).
"""
import sys
sys.path.insert(0, '/opt/trn_rl_repo')
import numpy as np

N_CORES = 8
B = 128
BC = B // N_CORES
L = 4096
NFFT = 256
NF = 129
T = 33
NE = 8

_cache = {}


def _interp_tables():
    coords = np.clip((np.arange(T) + 0.5) * (64.0 / T) - 0.5, 0.0, 63.0)
    lo = np.floor(coords).astype(np.int64)
    w = coords - lo
    runs = []
    a = 0
    while a < T:
        b = a + 1
        if b < T:
            step = lo[a + 1] - lo[a]
            while b < T and lo[b] - lo[b - 1] == step:
                b += 1
        runs.append((a, b - a, int(lo[a]), int(lo[a + 1] - lo[a]) if b - a >= 2 else 1))
        a = b
    return runs, lo, w


# const blob layouts: name -> (n_partitions, [free shape]); offsets assigned
# in declaration order, 1 col = 4 bytes on every partition. blobR items feed
# f32r matmuls (DMA'd with an f32r bitcast); blobF items are plain fp32.
BLOB_R0_ITEMS = [
    ('crw', 128, [2, NF]), ('ciw', 128, [2, NF]),
]
# bf16 consts for the line branch (conv1 + conv2 tap-packed pairs + conv3)
BLOB_B_ITEMS = [
    ('w15', 15, [128]),
    ('w2Lb', 128, [3, 128]),
    ('w3Lb', 128, [3, 256]),
]
BLOB_F_ITEMS = [
    ('gw1ta', 128, [128]), ('gw1tb', 1, [128]), ('gb1c', 128, [1]),
    ('gw2t', 128, [NE]), ('gb2c', NE, [1]), ('iota8', BC, [NE]),
    ('ones1', 1, [528]), ('sel8', NE, [NE * 128]),
    ('w1b', 128, [2, NE]), ('w2b', 128, [2, NE]),
    ('lb1c', 128, [1]),
    ('lb2c', 128, [1]),
    ('se2w1t', 128, [32]), ('se2w2t', 32, [128]),
    ('lb3c', 128, [2, 1]),
    ('se3w1t', 128, [2, 64]), ('se3w2t', 64, [256]),
    ('wlo', 1, [T]),
    ('negone', 128, [1]),
    ('i16', 16, [16]), ('i128', 128, [128]),
    ('bfT', 128, [8]), ('bbT', 128, [8]),
    ('ffn1t', 128, [4, 256]), ('ffnb1', 128, [2, 1]), ('ffn2t', 128, [2, 1]),
]


def _blob_layout(items):
    off = 0
    lay = {}
    for name, p, shape in items:
        w = int(np.prod(shape))
        lay[name] = (off, p, shape, w)
        off += w
    return lay, off


def _build(ffn_b2_val):
    from concourse import bass, tile, mybir
    from concourse.mybir import AluOpType as alu
    from concourse.mybir import ActivationFunctionType as actf

    f32 = mybir.dt.float32
    f32r = mybir.dt.float32r
    X = mybir.AxisListType.X

    nc = bass.Bass()
    bf16 = mybir.dt.bfloat16
    inp = lambda name, shape: nc.declare_dram_parameter(name, list(shape), f32, isOutput=False)

    layR0, blobR0_w = _blob_layout(BLOB_R0_ITEMS)
    layB, blobB_w = _blob_layout(BLOB_B_ITEMS)
    layF, blobF_w = _blob_layout(BLOB_F_ITEMS)
    d = {}
    for name, shape in [
        ('blobR0', [128, blobR0_w]),
        ('blobF', [128, blobF_w]),
        ('framesT', [NFFT, BC * T]),
        ('w1sb', [128, 48, 256]), ('w2sb', [128, 48, 256]),
        ('wihftT', [128, 4, 1024]), ('wihbtT', [128, 4, 1024]),
    ]:
        d[name] = inp(name, shape)
    d['whhbfT'] = nc.declare_dram_parameter('whhbfT', [128, 2, 1024], bf16, isOutput=False)
    d['blobB'] = nc.declare_dram_parameter('blobB', [128, blobB_w], bf16, isOutput=False)
    d['x14d'] = nc.declare_dram_parameter('x14d', [15, 8, L], bf16, isOutput=False)
    yout = nc.declare_dram_parameter('yout', [BC], f32, isOutput=True)

    runs, lo_t, w_t = _interp_tables()

    def mm(out, lhsT, rhs, start, stop):
        nc.tensor.matmul(out, lhsT, rhs, start=start, stop=stop)

    def mmr(out, lhsT, rhs, start, stop):
        nc.tensor.matmul(out, lhsT.bitcast(f32r), rhs.bitcast(f32r),
                         start=start, stop=stop)

    def mmf(out, lhsT, rhs, start, stop):
        nc.tensor.matmul(out, lhsT, rhs, start=start, stop=stop)

    with tile.TileContext(nc, num_cores=N_CORES) as tc:
        with (
            tc.tile_pool(name='const', bufs=1) as cp,
            tc.tile_pool(name='work', bufs=2) as wp,
            tc.tile_pool(name='one', bufs=1) as wp1,
            tc.tile_pool(name='big', bufs=1) as bp,
            tc.tile_pool(name='psA', bufs=2, space='PSUM') as psA,
            tc.tile_pool(name='psB', bufs=6, space='PSUM') as psB,
        ):
            # DMA order sets HWDGE/DMA-device queue order: framesT + small
            # f32r consts first so STFT/conv1 start early; the rest follow.
            c_fr0 = cp.tile([128, 2, BC * T], f32, tag='framesT', name='framesT')
            nc.sync.dma_start(c_fr0[:].bitcast(f32r),
                              d['framesT'][:].rearrange('(k p) m -> p k m', p=128).bitcast(f32r))
            blobR0 = cp.tile([128, blobR0_w], f32, tag='blobR0')
            nc.sync.dma_start(blobR0[:].bitcast(f32r), d['blobR0'][:].bitcast(f32r))
            blobF = cp.tile([128, blobF_w], f32, tag='blobF')
            nc.sync.dma_start(blobF[:], d['blobF'][:])
            blobB = cp.tile([128, blobB_w], bf16, tag='blobB')
            nc.sync.dma_start(blobB[:], d['blobB'][:])
            whbf = cp.tile([128, 2, 1024], bf16, tag='whbf')
            nc.sync.dma_start(whbf[:], d['whhbfT'][:])
            C = {}
            for blob, lay in ((blobR0, layR0), (blobB, layB), (blobF, layF)):
                for name, (off, p, shape, w) in lay.items():
                    v = blob[0:p, off:off + w]
                    if len(shape) > 1:
                        v = v.rearrange('p (a b) -> p a b', b=shape[-1])
                    C[name] = v

            def zero_r(view, shape):
                # zeros tagged f32r (memset can't emit f32r): const * 0.0
                src = C['gb1c'][:, 0:1]
                for _ in range(len(shape) - 2):
                    src = src.unsqueeze(2)
                nc.vector.tensor_scalar_mul(view.bitcast(f32r),
                                            src.to_broadcast(list(shape)), 0.0)

            ep_cm = tc.tile_pool(name='ep', bufs=2)
            ep = ep_cm.__enter__()
            c_fr = c_fr0

            # ---------------- line conv1 (host im2col, 8 chunk DMAs) --------
            # emission is interleaved into the expert loop below so the DVE
            # maxpool work overlaps with the PE-bound expert matmuls.
            # Bias rides in the matmul (w15 row 14 x ones row), so
            # relu folds into the final pool stage (max with 0) and h1 is
            # bf16, ready for the tap-packed bf16 conv2.
            h1 = bp.tile([128, 8, 1028], bf16, tag='h1', name='h1')
            nc.vector.memset(h1[:, :, 0:2], 0.0)
            nc.vector.memset(h1[:, :, 1026:1028], 0.0)
            # tap-stacked conv2 inputs: hstk_s rows 0:64 = h1 parity-s rows,
            # rows 64:128 = same rows shifted one column (partition moves are
            # DMA-only). relu + DMA stream out per finished column pair so
            # nothing serializes behind the expert loop.
            hstk = [bp.tile([128, 8, 1028], bf16, tag=f'hstk{s}', name=f'hstk{s}')
                    for s in range(2)]
            # conv2-line state lives up here: the uh=0 half interleaves into
            # expert iterations 6-7 (its hstk columns complete at chunk 5)
            hp2 = bp.tile([128, BC, 258], bf16, tag='hp2', name='hp2')
            nc.vector.memset(hp2[:, :, 0:1], 0.0)
            nc.vector.memset(hp2[:, :, 257:258], 0.0)
            hp2v = hp2[:, :, 1:257].rearrange('p (g s) t -> p s g t', s=2)
            seacc = bp.tile([128, 2, 8, 2], f32, tag='seacc', name='seacc')
            conv2_state = {'i': 0, 'pool': None}
            CONV2_UNITS = [(s, gg, uh) for uh in range(2) for s in range(2)
                           for gg in range(8)]

            def emit_conv2(n):
                pool = conv2_state['pool']
                for _ in range(n):
                    i = conv2_state['i']
                    if i >= len(CONV2_UNITS):
                        return
                    s, gg, uh = CONV2_UNITS[i]
                    pl2 = psB.tile([128, 512], f32, tag='pb', name='pb')
                    for j, c0 in enumerate((0, 1, 4)):
                        kn = 128 if j < 2 else 64
                        mm(pl2[:], C['w2Lb'][0:kn, j, :],
                           hstk[s][0:kn, gg, uh * 512 + c0:uh * 512 + c0 + 512],
                           j == 0, j == 2)
                    r2 = pool.tile([128, 128, 4], bf16, tag='r2', name='r2', bufs=2)
                    nc.scalar.activation(r2[:], pl2[:].rearrange('p (w q) -> p w q', q=4),
                                         actf.Relu, bias=C['lb2c'][:, 0:1],
                                         accum_out=seacc[:, s, gg, uh:uh + 1])
                    m2 = pool.tile([128, 128, 2], bf16, tag='m2b', name='m2b', bufs=2)
                    nc.vector.tensor_tensor(m2[:], r2[:, :, 0:2], r2[:, :, 2:4],
                                            alu.max)
                    nc.vector.tensor_tensor(hp2v[:, s, gg, uh * 128:uh * 128 + 128],
                                            m2[:, :, 0], m2[:, :, 1], alu.max)
                    conv2_state['i'] = i + 1
            NCH = 8
            CW = L // NCH  # 512
            line_state = {'i': 0, 'x14': None, 'pre': -1}

            def line_flush_pair(q):
                # chunks q-1, q are complete: relu in place, then copy into
                # the tap-stacked layout (cols P0..P1). High rows are shifted
                # by TWO columns (keeps bf16 DMA offsets 4-byte aligned);
                # taps pair as (0,2), (1,3), (4).
                P0 = 2 + (q - 1) * 128
                P1 = 2 + (q + 1) * 128
                lo = 0 if q == 1 else P0
                hi = 1028 if q == NCH - 1 else P1
                nc.vector.tensor_scalar_max(h1[:, :, P0:P1], h1[:, :, P0:P1], 0.0)
                for s in range(2):
                    rows = slice(s * 64, s * 64 + 64)
                    nc.sync.dma_start(hstk[s][0:64, :, lo:hi], h1[rows, :, lo:hi])
                    src0 = max(lo, 2)
                    nc.sync.dma_start(hstk[s][64:128, :, src0 - 2:hi - 2],
                                      h1[rows, :, src0:hi])

            def line_dma(q):
                x14 = ep.tile([15, 8, CW], bf16, tag='x14', name='x14')
                nc.sync.dma_start(x14[:], d['x14d'][:, :, q * CW:(q + 1) * CW])
                line_state['x14'] = x14
                line_state['pre'] = q

            line_dma(0)  # chunk-0 input prefetched ahead of the other DMAs

            def emit_line(n):
                for _ in range(n):
                    i = line_state['i']
                    if i >= NCH * 8:
                        return
                    q, gg = divmod(i, 8)
                    if gg == 0 and line_state['pre'] != q:
                        line_dma(q)
                    x14 = line_state['x14']
                    pl1 = psB.tile([128, CW], f32, tag='pb', name='pb')
                    mm(pl1[:], C['w15'][:], x14[:, gg, :], True, True)
                    o0 = 2 + (q * CW) // 4
                    # bf16-out reduce (DVE can read PSUM only once per op);
                    # relu = max(.,0) runs as a 4x-mode bf16 tensor_scalar
                    # per gg after the loop (bias already in the matmul)
                    nc.vector.tensor_reduce(h1[:, gg, o0:o0 + CW // 4],
                                            pl1[:].rearrange('p (u q) -> p u q', q=4),
                                            X, alu.max)
                    line_state['i'] = i + 1
                    if line_state['i'] % 8 == 0:
                        if q + 1 < NCH:
                            line_dma(q + 1)  # prefetch next chunk's input
                        if q % 2 == 1:
                            line_flush_pair(q)
            # ---------------- STFT magnitude ----------------
            magA = ep.tile([128, BC, T + 4], f32, tag='magA', name='magA', bufs=1)
            magB = ep.tile([1, BC, T + 4], f32, tag='magB', name='magB', bufs=1)
            nc.vector.memset(magA[:], 0.0)
            nc.vector.memset(magB[:], 0.0)
            NB2 = BC * T // 2
            for m0, mn, magX in [(0, 128, magA), (128, 1, magB)]:
                sqr = ep.tile([mn, BC * T], f32, tag='sqr', name=f'sqr{m0}', bufs=1)
                sqi = ep.tile([mn, BC * T], f32, tag='sqi', name=f'sqi{m0}', bufs=1)
                for ni in range(2):
                    pre = psA.tile([mn, NB2], f32, tag='pa', name='pa')
                    pim = psA.tile([mn, NB2], f32, tag='pa', name='pa')
                    for k in range(2):
                        co = slice(ni * NB2, (ni + 1) * NB2)
                        mmr(pre[:], C['crw'][:, k, m0:m0 + mn], c_fr[:, k, co], k == 0, k == 1)
                        mmr(pim[:], C['ciw'][:, k, m0:m0 + mn], c_fr[:, k, co], k == 0, k == 1)
                    nc.scalar.square(sqr[:, ni * NB2:(ni + 1) * NB2], pre[:])
                    nc.scalar.square(sqi[:, ni * NB2:(ni + 1) * NB2], pim[:])
                nc.vector.tensor_add(sqr[:], sqr[:], sqi[:])
                nc.scalar.sqrt(magX[0:mn, :, 2:2 + T],
                               sqr[:].rearrange('p (b t) -> p b t', b=BC))


            # ---------------- gating (fp32 matmuls, exact top-k) ----------------
            pooledA = ep.tile([128, BC], f32, tag='pooledA', name='pooledA')
            pooledB = ep.tile([1, BC], f32, tag='pooledB', name='pooledB')
            nc.vector.tensor_reduce(pooledA[:], magA[:, :, 2:2 + T], X, alu.add)
            nc.vector.tensor_reduce(pooledB[:], magB[:, :, 2:2 + T], X, alu.add)
            pg1 = psA.tile([128, BC], f32, tag='pa', name='pa')
            mmf(pg1[:], C['gw1ta'][:], pooledA[:], True, False)
            mmf(pg1[:], C['gw1tb'][:], pooledB[:], False, True)
            gh = ep.tile([128, BC], f32, tag='gh', name='gh')
            nc.scalar.activation(gh[:], pg1[:], actf.Relu, bias=C['gb1c'][:, 0:1])
            pg2 = psA.tile([NE, BC], f32, tag='pa', name='pa')
            mmf(pg2[:], C['gw2t'][:], gh[:], True, True)
            logitsT = ep.tile([NE, BC], f32, tag='logitsT', name='logitsT')
            nc.vector.tensor_tensor(logitsT[:], pg2[:],
                                    C['gb2c'][:, 0:1].to_broadcast([NE, BC]), alu.add)
            plg = psA.tile([BC, NE], f32, tag='pa', name='pa')
            nc.tensor.transpose(plg[:], logitsT[:], C['i16'][0:NE, 0:NE])
            lg = ep.tile([BC, NE], f32, tag='lg', name='lg')
            nc.vector.tensor_copy(lg[:], plg[:])
            iob = C['iota8']

            def small(tag, shape=(BC, NE)):
                return ep.tile(list(shape), f32, tag=tag, name=tag)

            m1 = small('m1', (BC, 1))
            nc.vector.tensor_reduce(m1[:], lg[:], X, alu.max)
            eq1 = small('eq1')
            nc.vector.tensor_tensor(eq1[:], lg[:], m1[:].to_broadcast([BC, NE]), alu.is_equal)
            l2 = small('l2')
            nc.vector.scalar_tensor_tensor(l2[:], eq1[:], -1e30, lg[:], alu.mult, alu.add)
            m2 = small('m2', (BC, 1))
            nc.vector.tensor_reduce(m2[:], l2[:], X, alu.max)
            it1 = small('it1')
            nc.vector.tensor_tensor(it1[:], eq1[:], iob, alu.mult)
            idx1 = small('idx1', (BC, 1))
            nc.vector.tensor_reduce(idx1[:], it1[:], X, alu.max)
            eq2 = small('eq2')
            nc.vector.tensor_tensor(eq2[:], l2[:], m2[:].to_broadcast([BC, NE]), alu.is_equal)
            it2 = small('it2')
            nc.vector.tensor_tensor(it2[:], eq2[:], iob, alu.mult)
            idx2 = small('idx2', (BC, 1))
            nc.vector.tensor_reduce(idx2[:], it2[:], X, alu.max)
            dm = small('dm', (BC, 1))
            nc.vector.tensor_sub(dm[:], m1[:], m2[:])
            g1 = small('g1', (BC, 1))
            nc.scalar.activation(g1[:], dm[:], actf.Sigmoid)
            g2 = small('g2', (BC, 1))
            nc.vector.tensor_scalar(g2[:], g1[:], -1.0, 1.0, alu.mult, alu.add)
            eA = small('eA')
            nc.vector.tensor_tensor(eA[:], idx1[:].to_broadcast([BC, NE]), iob, alu.is_equal)
            eB = small('eB')
            nc.vector.tensor_tensor(eB[:], idx2[:].to_broadcast([BC, NE]), iob, alu.is_equal)
            tA = small('tA')
            nc.vector.tensor_tensor(tA[:], eA[:], g1[:].to_broadcast([BC, NE]), alu.mult)
            tB = small('tB')
            nc.vector.tensor_tensor(tB[:], eB[:], g2[:].to_broadcast([BC, NE]), alu.mult)
            W8 = small('W8')
            nc.vector.tensor_add(W8[:], tA[:], tB[:])
            pW8T = psA.tile([NE, BC], f32, tag='pa', name='pa')
            nc.tensor.transpose(pW8T[:], W8[:], C['i16'][:])
            W8T = ep.tile([NE, BC], f32, tag='W8T', name='W8T')
            nc.vector.tensor_copy(W8T[:], pW8T[:])

            # (h1 relu is emitted after the expert loop: the in-order Act
            # queue would otherwise stall expert activations for ~8us)

            # ---------------- experts (dense, weighted sum) ----------------
            imt = [ep.tile([128 if k < 5 else 5, BC, T], f32, tag=f'im1_{k}', name=f'im1_{k}', bufs=1)
                   for k in range(6)]
            wts = {}

            def dma_weights(e):
                w1s = ep.tile([128, 6, 256], f32, tag='w1s', name='w1s')
                nc.sync.dma_start(w1s[:, 0:5, :].bitcast(f32r),
                                  d['w1sb'][:, e * 6:e * 6 + 5, :].bitcast(f32r))
                nc.sync.dma_start(w1s[0:5, 5, :].bitcast(f32r),
                                  d['w1sb'][0:5, e * 6 + 5, :].bitcast(f32r))
                w2s = ep.tile([128, 6, 256], f32, tag='w2s', name='w2s')
                nc.sync.dma_start(w2s[:].bitcast(f32r),
                                  d['w2sb'][:, e * 6:(e + 1) * 6, :].bitcast(f32r))
                wts[e] = (w1s, w2s)

            dma_weights(0)  # expert-0 weights ahead of the imt copies
            for dt in range(5):
                pos = dt * NF
                done = 0
                while done < NF:
                    k, r = divmod(pos + done, 128)
                    if done < 128:
                        n = min(128 - r, NF - done, 128 - done)
                        nc.sync.dma_start(imt[k][r:r + n].bitcast(f32r),
                                          magA[done:done + n, :, dt:dt + T].bitcast(f32r))
                    else:
                        n = 1
                        nc.sync.dma_start(imt[k][r:r + 1].bitcast(f32r), magB[0:1, :, dt:dt + T].bitcast(f32r))
                    done += n
            accF = [bp.tile([128, BC, T], f32, tag=f'accF{i}', name=f'accF{i}') for i in range(2)]
            accP = [bp.tile([128, BC, T], f32, tag=f'accP{i}', name=f'accP{i}') for i in range(2)]
            H = BC // 2
            # persistent he pairs, double-buffered by expert parity so
            # expert e+1's conv1 activations don't wait on expert e's conv2
            he_sets = [[bp.tile([128, BC * (T + 2) + 2], f32,
                                tag=f'he_{s}_{i}', name=f'he_{s}_{i}')
                        for i in range(2)] for s in range(2)]
            for he in he_sets:
                for i in range(2):
                    hv = he[i][:, 0:BC * (T + 2)].rearrange('p (b t) -> p b t', t=T + 2)
                    zero_r(hv[:, :, 0:1], (128, BC, 1))
                    zero_r(hv[:, :, T + 1:T + 2], (128, BC, 1))
                    zero_r(he[i][:, BC * (T + 2):], (128, 2))
            conv2_state['pool'] = ep
            for e in range(NE):
                emit_line(2)
                if e >= 6:
                    emit_conv2(2)
                if e + 1 < NE:
                    dma_weights(e + 1)
                w1s, w2s = wts.pop(e)
                he = he_sets[e % 2]
                for mi in range(2):
                    for ni in range(2):
                        if mi == 1 and ni == 0:
                            emit_line(2)
                            if e >= 6:
                                emit_conv2(2)
                        pe1 = psB.tile([128, H * T], f32, tag='pb', name='pb')
                        for k in range(6):
                            kn = 128 if k < 5 else 5
                            mmr(pe1[:], w1s[0:kn, k, mi * 128:(mi + 1) * 128],
                               imt[k][:].rearrange('p b t -> p (b t)')[:, ni * H * T:(ni + 1) * H * T],
                               k == 0, k == 5)
                        nc.scalar.activation(he[mi][:, 0:BC * (T + 2)].rearrange('p (b t) -> p b t', t=T + 2)[:, ni * H:(ni + 1) * H, 1:1 + T].bitcast(f32r),
                                             pe1[:].rearrange('p (b t) -> p b t', t=T),
                                             actf.Relu, bias=C['w1b'][:, mi, e:e + 1])
                eo = [ep.tile([128, BC, T], f32, tag=f'eo{i}', name=f'eo{i}', bufs=3) for i in range(2)]
                W2 = T + 2
                emit_line(2)
                if e >= 6:
                    emit_conv2(2)
                for mi in range(2):
                    for bi in range(2):
                        if mi == 1 and bi == 0:
                            emit_line(2)
                            if e >= 6:
                                emit_conv2(2)
                        pe2 = psB.tile([128, H * W2], f32, tag='pb', name='pb')
                        for k in range(6):
                            dt, ch = divmod(k, 2)
                            mmr(pe2[:], w2s[:, k, mi * 128:(mi + 1) * 128],
                               he[ch][:, bi * H * W2 + dt:bi * H * W2 + dt + H * W2],
                               k == 0, k == 5)
                        nc.scalar.activation(eo[mi][:, bi * H:(bi + 1) * H, :],
                                             pe2[:].rearrange('p (b t) -> p b t', t=W2)[:, :, 0:T],
                                             actf.Relu, bias=C['w2b'][:, mi, e:e + 1])
                pwe = psA.tile([128, BC], f32, tag='pa', name='pwe')
                mmf(pwe[:], C['sel8'][:, e * 128:(e + 1) * 128], W8T[:], True, True)
                wE = ep.tile([128, BC], f32, tag='wE', name='wE', bufs=8)
                nc.scalar.copy(wE[:], pwe[:])
                wbc = wE[:].unsqueeze(2).to_broadcast([128, BC, T])
                # weighted accumulation split: DVE chain (even e) + Pool chain (odd e)
                for mi in range(2):
                    accX = accF[mi] if e % 2 == 0 else accP[mi]
                    eng = nc.vector if (e % 2 == 0) else nc.gpsimd
                    if e < 2:
                        eng.tensor_tensor(accX[:], eo[mi][:], wbc, alu.mult)
                    else:
                        eow = ep.tile([128, BC, T], f32, tag=f'eow{e % 2}', name=f'eow{e % 2}')
                        eng.tensor_tensor(eow[:], eo[mi][:], wbc, alu.mult)
                        eng.tensor_add(accX[:], accX[:], eow[:])
            emit_line(NCH * 8)  # flush any remaining line-conv1 units
            for mi in range(2):
                nc.vector.tensor_add(accF[mi][:], accF[mi][:], accP[mi][:])

            ep_cm.__exit__(None, None, None)

            lp_cm = tc.tile_pool(name='lp', bufs=2)
            lp = lp_cm.__enter__()

            # ---------------- line branch conv2/conv3/SE ----------------
            # conv2 tap-packed: 3 bf16 matmuls per (s,gg,uh) against hstk;
            # the uh=0 half already ran inside expert iterations 6-7
            conv2_state['pool'] = lp
            emit_conv2(len(CONV2_UNITS))
            seY = lp.tile([128, 2, 8], f32, tag='seY', name='seY')
            nc.vector.tensor_reduce(seY[:], seacc[:], X, alu.add)
            pse1 = psA.tile([32, 16], f32, tag='pa', name='pa')
            mmf(pse1[:], C['se2w1t'][:], seY[:].rearrange('p s g -> p (s g)'), True, True)
            z2 = lp.tile([32, 16], f32, tag='z2', name='z2')
            nc.scalar.activation(z2[:], pse1[:], actf.Relu)
            pse2 = psA.tile([128, 16], f32, tag='pa', name='pa')
            mmf(pse2[:], C['se2w2t'][:], z2[:], True, True)
            sc2 = lp.tile([128, 2, 8], bf16, tag='sc2', name='sc2')
            nc.scalar.activation(sc2[:].rearrange('p s g -> p (s g)'), pse2[:], actf.Sigmoid)
            for gg in range(8):
                nc.vector.tensor_tensor(
                    hp2[:, :, 1:257].rearrange('p (g s) t -> p g s t', s=2)[:, gg],
                    hp2[:, :, 1:257].rearrange('p (g s) t -> p g s t', s=2)[:, gg],
                    sc2[:].rearrange('p s g -> p g s')[:, gg].unsqueeze(2).to_broadcast([128, 2, 256]),
                    alu.mult)

            # ---- LSTM input precompute pass A (accF k-chunks, hidden under
            # conv2): XsT = bias + wih[k01]^T x01; pass B adds the li part.
            XsT = bp.tile([128, 8, 528], bf16, tag='XsT', name='XsT')
            wft = lp.tile([128, 4, 1024], f32, tag='wft', name='wft', bufs=1)
            nc.sync.dma_start(wft[:].bitcast(f32r), d['wihftT'][:].bitcast(f32r))
            xt_all = [[(lp if half == 0 else bp).tile(
                           [128, tl * 16], f32, tag=f'xt{half}_{k}',
                           name=f'xt{half}_{k}', bufs=1)
                       for k in range(4)]
                      for half, (t0, tl) in enumerate([(0, 16), (16, 17)])]
            for half, (t0, tl) in enumerate([(0, 16), (16, 17)]):
                for k in range(2):
                    nc.vector.tensor_copy(
                        xt_all[half][k][:].rearrange('p (t b) -> p t b', b=16).bitcast(f32r),
                        accF[k][:, :, t0:t0 + tl].transpose([0, 2, 1]))
            # k01 matmuls for half 1 only; half 0 is done in one full pass
            # right after the interp so the LSTM can start immediately
            t0, tl = 16, 17
            for j in range(8):
                px = psB.tile([128, tl * 16], f32, tag='pb', name='pb')
                for k in range(2):
                    mmr(px[:], wft[:, k, j * 128:(j + 1) * 128],
                        xt_all[1][k][:], k == 0, k == 1)
                # on DVE: Act is the bottleneck engine in this window
                nc.vector.tensor_scalar_add(XsT[:, j, t0 * 16:(t0 + tl) * 16],
                                            px[:], C['bfT'][:, j:j + 1])

            # conv3 + SE3 + pool, sample pairs batched into one matmul
            y3 = lp.tile([128, 2, BC], f32, tag='y3', name='y3')
            lfm = lp.tile([128, 2, BC, 64], bf16, tag='lfm', name='lfm', bufs=1)
            for mi in range(2):
                for bp2 in range(BC // 2):
                    b0 = bp2 * 2
                    pl3 = psB.tile([128, 2, 256], f32, tag='pb', name='pb')
                    for dt in range(3):
                        mm(pl3[:], C['w3Lb'][:, dt, mi * 128:(mi + 1) * 128],
                           hp2[:, b0:b0 + 2, dt:dt + 256], dt == 0, dt == 2)
                    r3 = lp.tile([128, 2, 64, 4], bf16, tag='r3', name='r3', bufs=3)
                    # balance the per-pair tail: Act sums sample b0 (accum),
                    # DVE sums sample b1
                    nc.scalar.activation(r3[:, 0], pl3[:, 0, :].rearrange('p (u q) -> p u q', q=4),
                                         actf.Relu, bias=C['lb3c'][:, mi, 0:1],
                                         accum_out=y3[:, mi, b0:b0 + 1])
                    nc.scalar.activation(r3[:, 1], pl3[:, 1, :].rearrange('p (u q) -> p u q', q=4),
                                         actf.Relu, bias=C['lb3c'][:, mi, 0:1])
                    nc.vector.tensor_reduce(y3[:, mi, b0 + 1:b0 + 2],
                                            r3[:, 1].rearrange('p u q -> p (u q)'),
                                            X, alu.add)
                    m3 = lp.tile([128, 2, 64, 2], bf16, tag='m3', name='m3', bufs=3)
                    nc.vector.tensor_tensor(m3[:], r3[:, :, :, 0:2], r3[:, :, :, 2:4],
                                            alu.max)
                    nc.vector.tensor_tensor(lfm[:, mi, b0:b0 + 2, :],
                                            m3[:, :, :, 0], m3[:, :, :, 1], alu.max)
            pse3 = psA.tile([64, 16], f32, tag='pa', name='pa')
            for k in range(2):
                mmf(pse3[:], C['se3w1t'][:, k, :], y3[:, k, :], k == 0, k == 1)
            z3 = lp.tile([64, 16], f32, tag='z3', name='z3')
            nc.scalar.activation(z3[:], pse3[:], actf.Relu)
            sc3 = lp.tile([128, 2, BC], f32, tag='sc3', name='sc3', bufs=1)
            for mi in range(2):
                pse4 = psA.tile([128, 16], f32, tag='pa', name='pa')
                mmf(pse4[:], C['se3w2t'][:, mi * 128:(mi + 1) * 128], z3[:], True, True)
                nc.scalar.activation(sc3[:, mi, :], pse4[:], actf.Sigmoid)
            # interp 64 -> 33 (both mi halves per op); the SE3 scale is
            # linear so it is applied after the lerp on the smaller tensor

            li = bp.tile([128, 2, BC, T], f32, tag='li', name='li')
            pwl = psA.tile([128, T], f32, tag='pa', name='pwl')
            mmf(pwl[:], C['ones1'][0:1, 0:128], C['wlo'][:], True, True)
            wlo128 = lp.tile([128, T], bf16, tag='wlo128', name='wlo128', bufs=1)
            nc.vector.tensor_copy(wlo128[:], pwl[:])
            wbc_all = wlo128[:]

            def emit_run(a, n, lo0, st):
                end = lo0 + (n - 1) * st + 1
                lov = lfm[:, :, :, lo0:end:st]
                hiv = lfm[:, :, :, lo0 + 1:end + 1:st]
                dd = lp.tile([128, 2, BC, T], bf16, tag='dd', name='dd')
                nc.vector.tensor_sub(dd[:, :, :, a:a + n], hiv, lov)
                nc.vector.tensor_tensor(dd[:, :, :, a:a + n], dd[:, :, :, a:a + n],
                                        wbc_all[:, a:a + n].unsqueeze(1).unsqueeze(1).to_broadcast([128, 2, BC, n]),
                                        alu.mult)
                nc.vector.tensor_add(li[:, :, :, a:a + n], dd[:, :, :, a:a + n], lov)

            # runs covering t<16 first: DVE is in-order, so the xt-h0 copies
            # (which only need li[..., 0:16]) must not queue behind the rest
            for (a, n, lo0, st) in runs:
                if a < 16:
                    emit_run(a, n, lo0, st)

            # ---- precompute finish, interleaved with the LSTM ---------------
            # half 0: full 4-matmul groups + Act copy (no DVE adds) so the
            # LSTM starts right after; half 1's k23 term lands during the
            # early LSTM steps (XsT dependencies are region-exact).
            for k in range(2, 4):
                nc.vector.tensor_tensor(
                    xt_all[0][k][:].rearrange('p (t b) -> p t b', b=16).bitcast(f32r),
                    li[:, k - 2, :, 0:16].transpose([0, 2, 1]),
                    sc3[:, k - 2, :].unsqueeze(1).to_broadcast([128, 16, 16]),
                    alu.mult)
            for (a, n, lo0, st) in runs:
                if a >= 16:
                    emit_run(a, n, lo0, st)
            for j in range(8):
                px = psB.tile([128, 256], f32, tag='pb', name='pb')
                for k in range(4):
                    mmr(px[:], wft[:, k, j * 128:(j + 1) * 128],
                        xt_all[0][k][:], k == 0, k == 3)
                nc.scalar.activation(XsT[:, j, 0:256], px[:],
                                     actf.Identity, bias=C['bfT'][:, j:j + 1])

            # ---------------- forward LSTM (33 steps, transposed) ------------
            # gate tanh folded into the single sigmoid (tanh x = 2*sig(2x)-1;
            # g-gate rows host-scaled by 2): one Act op for all 8 gate chunks.
            # Cell state kept as S = c/2 + 0.5 so the whole c-update is two
            # scalar_tensor_tensor ops:
            #   prod = (csgc[6:10] - 0.5) * csgc[0:4] = [i*tg/2, f*c/2]
            #   S'   = (prod[0:2] + 0.5) + prod[2:4]
            #   tanh(c) = tanh(2*S - 1) via Act scale/bias.
            # csgc: 0-7 = gates (i,i,f,f,o,o,g,g), 8-9 = S, 10-11 = tanh(c)
            csgc = bp.tile([128, 12, 16], bf16, tag='csgc', name='csgc')
            hT = bp.tile([128, 2, 16], bf16, tag='hT', name='hT')
            nc.vector.memset(csgc[:, 8:10, :], 0.5)
            nc.vector.memset(hT[:], 0.0)
            ib128 = wp1.tile([128, 128], bf16, tag='ib128', name='ib128')
            nc.vector.tensor_copy(ib128[:], C['i128'])

            def emit_step(t):
                gT = psB.tile([128, 8, 16], f32, tag='pb', name='pb')
                mm(gT[:], ib128[:], XsT[:, :, t * 16:(t + 1) * 16],
                   True, t == 0)
                if t > 0:
                    for j in range(8):
                        for k in range(2):
                            mm(gT[:, j, :], whbf[:, k, j * 128:(j + 1) * 128],
                               hT[:, k, :], False, k == 1)
                nc.scalar.activation(csgc[:, 0:8, :], gT[:], actf.Sigmoid)
                prod = wp.tile([128, 4, 16], bf16, tag='prod', name='prod', bufs=4)
                nc.vector.scalar_tensor_tensor(prod[:], csgc[:, 6:10, :], -0.5,
                                               csgc[:, 0:4, :], alu.add, alu.mult)
                nc.vector.scalar_tensor_tensor(csgc[:, 8:10, :], prod[:, 0:2, :], 0.5,
                                               prod[:, 2:4, :], alu.add, alu.add)
                nc.scalar.activation(csgc[:, 10:12, :], csgc[:, 8:10, :], actf.Tanh,
                                     bias=C['negone'][:, 0:1], scale=2.0)
                nc.vector.tensor_tensor(hT[:], csgc[:, 4:6, :],
                                        csgc[:, 10:12, :], alu.mult)

            for t in range(3):
                emit_step(t)

            # half 1's k23 term (psA banks; DVE adds fill LSTM idle slots)
            for k in range(2, 4):
                nc.vector.tensor_tensor(
                    xt_all[1][k][:].rearrange('p (t b) -> p t b', b=16).bitcast(f32r),
                    li[:, k - 2, :, 16:33].transpose([0, 2, 1]),
                    sc3[:, k - 2, :].unsqueeze(1).to_broadcast([128, 17, 16]),
                    alu.mult)
            for j in range(8):
                px = psA.tile([128, 272], f32, tag='pa', name='pa')
                for k in range(2, 4):
                    mmr(px[:], wft[:, k, j * 128:(j + 1) * 128],
                        xt_all[1][k][:], k == 2, k == 3)
                nc.vector.tensor_add(XsT[:, j, 256:528],
                                     XsT[:, j, 256:528], px[:])

            lp_cm.__exit__(None, None, None)
            xt_h2 = xt_all[1]

            # ---------------- backward cell (t=32, transposed) --------------
            xp_cm = tc.tile_pool(name='xp', bufs=2)
            xpp = xp_cm.__enter__()
            wbt = xpp.tile([128, 4, 1024], f32, tag='wbt', name='wbt', bufs=1)
            nc.sync.dma_start(wbt[:].bitcast(f32r), d['wihbtT'][:].bitcast(f32r))
            gbT = psA.tile([128, 8, 16], f32, tag='pa', name='gbT')
            for j in range(8):
                for k in range(4):
                    mmr(gbT[:, j, :], wbt[:, k, j * 128:(j + 1) * 128],
                        xt_h2[k][:].rearrange('p (t b) -> p t b', b=16)[:, 16, :],
                        k == 0, k == 3)
            sgb = wp1.tile([128, 8, 16], f32, tag='sgb', name='sgb')
            for j in range(8):
                fn = actf.Sigmoid if j < 6 else actf.Tanh
                nc.scalar.activation(sgb[:, j, :], gbT[:, j, :], fn, bias=C['bbT'][:, j:j + 1])
            cbT = wp1.tile([128, 2, 16], f32, tag='cbT', name='cbT')
            nc.vector.tensor_tensor(cbT[:], sgb[:, 0:2, :], sgb[:, 6:8, :], alu.mult)
            tcbT = wp1.tile([128, 2, 16], f32, tag='tcbT', name='tcbT')
            nc.scalar.activation(tcbT[:], cbT[:], actf.Tanh)
            hbT = bp.tile([128, 2, 16], f32, tag='hbT', name='hbT')
            nc.vector.tensor_tensor(hbT[:], sgb[:, 4:6, :], tcbT[:], alu.mult)
            xp_cm.__exit__(None, None, None)

            for t in range(3, T):
                emit_step(t)

            # ---------------- FFN head ----------------
            hTf = wp1.tile([128, 2, 16], f32, tag='hTf', name='hTf')
            nc.vector.tensor_copy(hTf[:], hT[:])
            lastT = [hTf[:, 0, :], hTf[:, 1, :], hbT[:, 0, :], hbT[:, 1, :]]
            z = [wp1.tile([128, 16], f32, tag=f'z_{i}', name=f'z_{i}') for i in range(2)]
            for mi in range(2):
                pz = psA.tile([128, 16], f32, tag='pa', name='pa')
                for k in range(4):
                    mmf(pz[:], C['ffn1t'][:, k, mi * 128:(mi + 1) * 128], lastT[k],
                        k == 0, k == 3)
                nc.scalar.activation(z[mi][:], pz[:], actf.Relu,
                                     bias=C['ffnb1'][:, mi, 0:1])
            py = psA.tile([1, 16], f32, tag='pa', name='pa')
            for k in range(2):
                mmf(py[:], C['ffn2t'][:, k, :], z[k][:], k == 0, k == 1)
            yo = wp1.tile([1, 16], f32, tag='yo', name='yo')
            nc.scalar.activation(yo[:], py[:], actf.Copy, bias=float(ffn_b2_val))
            nc.sync.dma_start(yout[:].unsqueeze(0), yo[:])

    _split_tpb_waits(nc)
    return nc


def _split_tpb_waits(nc, max_waits=1):
    """This walrus build caps sync-waits per TPB instruction; hoist extras
    onto same-engine NoOps placed immediately before the instruction."""
    from concourse import mybir
    cnt = 0
    for f in nc.m.functions:
        for bb in f.blocks:
            out = []
            changed = False
            for inst in bb.instructions:
                si = inst.sync_info
                if (si is not None and len(si.on_wait) > max_waits
                        and getattr(inst, 'engine', None) is not None):
                    waits = list(si.on_wait)
                    for w in waits[:-max_waits]:
                        nop = mybir.InstNoOp(name=f'{inst.name}-sw{cnt}', ins=[], outs=[])
                        cnt += 1
                        nop.engine = inst.engine
                        nop.sync_info = mybir.SyncInfo(on_wait=[w], on_update=[])
                        out.append(nop)
                    inst.sync_info = mybir.SyncInfo(on_wait=waits[-max_waits:],
                                                    on_update=list(si.on_update))
                    changed = True
                out.append(inst)
            if changed:
                bb.instructions = out
    return nc


def _host_prep(inputs):
    f = lambda x: np.ascontiguousarray(x, dtype=np.float32)
    n = np.arange(NFFT)
    win = 0.5 * (1.0 - np.cos(2.0 * np.pi * n / NFFT))
    k = np.arange(NF)
    ang = 2.0 * np.pi * np.outer(n, k) / NFFT
    gw1t = inputs['gate_w1'].T / T
    runs, lo_t, w_t = _interp_tables()
    w15 = np.zeros((15, 128), np.float32)
    for s in range(2):
        for jj in range(7):
            w15[s * 7 + jj, s * 64:(s + 1) * 64] = inputs['lw1'][:, 0, jj]
    w15[14, :] = np.tile(inputs['lb1'], 2)  # conv1 bias via ones row

    def kpm(arr, kk):
        # [kk*128, m] -> [128, kk, m] host-side '(k p) m -> p k m'
        m = arr.shape[-1]
        return np.ascontiguousarray(
            arr.reshape(kk, 128, m).transpose(1, 0, 2), dtype=np.float32)

    # gate reorder torch (i,f,g,o) -> (i,f,o,g) on the 1024-dim; forward
    # g-gate rows scaled by 2 (tanh x = 2*sig(2x)-1 trick)
    perm = np.r_[0:512, 768:1024, 512:768]
    gsc = np.ones((1024, 1), np.float32)
    gsc[768:1024] = 2.0
    whh_n = inputs['whh_f'][perm] * gsc    # [1024, 256]
    wih_n = inputs['wih_f'][perm] * gsc    # [1024, 512]
    bf_n = (inputs['bih_f'] + inputs['bhh_f'])[perm] * gsc[:, 0]
    wihb_n = inputs['wih_b'][perm]
    bb_n = (inputs['bih_b'] + inputs['bhh_b'])[perm]

    ones528 = np.ones((1, 528), np.float32)
    # tap-packed conv2 weights [128, 3, 128]: high rows see data shifted by
    # +2 columns, so pairs are (tap0,tap2)@0, (tap1,tap3)@1, (tap4)@4
    lw2t = np.transpose(inputs['lw2'], (1, 2, 0))                   # [64, 5, 128]
    w2Lb = np.zeros((128, 3, 128), np.float32)
    for j in range(2):
        w2Lb[0:64, j, :] = lw2t[:, j, :]
        w2Lb[64:128, j, :] = lw2t[:, j + 2, :]
    w2Lb[0:64, 2, :] = lw2t[:, 4, :]
    w3L_h = np.transpose(f(np.transpose(inputs['lw3'], (2, 1, 0))), (1, 0, 2))  # [128, 3, 256]
    sel8 = f(np.concatenate([np.tile(v[:, None], (1, 128)) for v in np.eye(NE)], axis=1))
    vals = {
        'crw': kpm(f(win[:, None] * np.cos(ang)), 2),
        'ciw': kpm(f(win[:, None] * np.sin(ang)), 2),
        'gw1ta': f(gw1t[0:128]), 'gw1tb': f(gw1t[128:129]),
        'gb1c': f(inputs['gate_b1'][:, None]),
        'gw2t': f(inputs['gate_w2'].T), 'gb2c': f(inputs['gate_b2'][:, None]),
        'iota8': f(np.tile(np.arange(NE)[None, :], (BC, 1))),
        'ones1': ones528, 'sel8': sel8,
        'w1b': kpm(f(np.ascontiguousarray(inputs['exp_b1'].T)), 2),
        'w2b': kpm(f(np.ascontiguousarray(inputs['exp_b2'].T)), 2),
        'w15': w15, 'lb1c': f(np.tile(inputs['lb1'], 2)[:, None]),
        'w2Lb': w2Lb,
        'lb2c': f(inputs['lb2'][:, None]),
        'se2w1t': f(inputs['se2_w1'].T / 1024.0), 'se2w2t': f(inputs['se2_w2'].T),
        'w3Lb': w3L_h,
        'lb3c': kpm(f(inputs['lb3'][:, None]), 2),
        'se3w1t': kpm(f(inputs['se3_w1'].T / 256.0), 2),
        'se3w2t': f(inputs['se3_w2'].T),
        'wlo': f(w_t[None, :]),
        'negone': np.full((128, 1), -1.0, np.float32),
        'i16': f(np.eye(16)), 'i128': f(np.eye(128)),
        'bfT': f(bf_n.reshape(8, 128).T),                          # [128, 8]
        'bbT': f(bb_n.reshape(8, 128).T),
        'ffn1t': kpm(f(np.ascontiguousarray(inputs['ffn_w1'].T)), 4),
        'ffnb1': kpm(f(inputs['ffn_b1'][:, None]), 2),
        'ffn2t': kpm(f(np.ascontiguousarray(inputs['ffn_w2'].T)), 2),
    }
    import ml_dtypes
    layR0, blobR0_w = _blob_layout(BLOB_R0_ITEMS)
    layB, blobB_w = _blob_layout(BLOB_B_ITEMS)
    layF, blobF_w = _blob_layout(BLOB_F_ITEMS)
    blobR0 = np.zeros((128, blobR0_w), np.float32)
    blobB = np.zeros((128, blobB_w), ml_dtypes.bfloat16)
    blobF = np.zeros((128, blobF_w), np.float32)
    for blob, lay in ((blobR0, layR0), (blobB, layB), (blobF, layF)):
        for name, (off, p, shape, w) in lay.items():
            v = np.asarray(vals[name], dtype=np.float32).reshape(p, w)
            blob[0:p, off:off + w] = v.astype(blob.dtype)

    # expert weights in SBUF layout [128, 48, 256]
    w1p = np.zeros((NE, 768, 256), np.float32)
    w1p[:, 0:645, :] = np.transpose(inputs['exp_w1'], (0, 3, 2, 1)).reshape(NE, 645, 256)
    w2p = np.transpose(inputs['exp_w2'], (0, 3, 2, 1)).reshape(NE, 768, 256)
    def esb(wp_):
        # [NE, 6*128, 256] -> [128, NE*6, 256]
        return np.ascontiguousarray(
            wp_.reshape(NE, 6, 128, 256).transpose(2, 0, 1, 3).reshape(128, 48, 256),
            dtype=np.float32)
    whhbfT = kpm(f(np.ascontiguousarray(whh_n.T)), 2).astype(ml_dtypes.bfloat16)
    shared = {
        'blobR0': blobR0, 'blobB': blobB, 'blobF': blobF,
        'whhbfT': whhbfT,
        'w1sb': esb(w1p), 'w2sb': esb(w2p),
        'wihftT': kpm(f(np.ascontiguousarray(wih_n.T)), 4),        # [128, 4, 1024]
        'wihbtT': kpm(f(np.ascontiguousarray(wihb_n.T)), 4),
    }

    xp = np.pad(inputs['x_continuum'], ((0, 0), (NFFT // 2, NFFT // 2)), mode='reflect')
    s0, s1 = xp.strides
    frames = np.lib.stride_tricks.as_strided(xp, (B, T, NFFT), (s0, 128 * s1, s1))
    xnp = np.pad(inputs['x_normalized'], ((0, 0), (3, 3)))
    in_maps = []
    for c in range(N_CORES):
        m = dict(shared)
        fr = frames[c * BC:(c + 1) * BC]
        m['framesT'] = f(np.transpose(fr, (2, 0, 1)).reshape(NFFT, BC * T))
        xc = xnp[c * BC:(c + 1) * BC]                               # [16, 4102]
        x14d = np.zeros((15, 8, L), np.float32)
        for s in range(2):
            for jj in range(7):
                x14d[s * 7 + jj] = xc[np.arange(8) * 2 + s][:, jj:jj + L]
        x14d[14] = 1.0  # ones row: conv1 bias via w15 row 14
        m['x14d'] = x14d.astype(ml_dtypes.bfloat16)
        in_maps.append(m)
    return in_maps


def _apply_tile_patch():
    from concourse import tile, mybir
    from concourse.vector_clock import ScopedClock

    def _drain_split(self, tick_clock, wait_clock):
        nc2 = self.nc
        di = nc2.sync.drain()
        wait_clock.add_sem_waits(di.ins, ScopedClock({None: tick_clock.global_clock}))
        si = di.ins.sync_info
        if si is not None and len(si.on_wait) > 1:
            waits = list(si.on_wait)
            ups = list(si.on_update)
            di.ins.sync_info = mybir.SyncInfo(on_wait=waits[:1], on_update=[])
            for kk, w in enumerate(waits[1:]):
                extra = nc2.sync.drain()
                extra.ins.sync_info = mybir.SyncInfo(
                    on_wait=[w], on_update=ups if kk == len(waits) - 2 else [])
        nc2.all_engine_barrier()
        assert self.sems is not None
        popped = nc2._tile_sem_poison_stack.pop()
        assert popped is self._sem_poison
        nc2.clear_and_free_semaphores(list(self.sems.allocated().values()))
        nc2.all_engine_barrier()

    tile.TileContext._drain_and_barrier = _drain_split


def kernel(**inputs):
    global _cache
    if 'nc' not in _cache:
        _apply_tile_patch()
        _cache['nc'] = _build(float(np.asarray(inputs['ffn_b2']).reshape(-1)[0]))
    from concourse.bass_utils import run_bass_kernel_spmd
    in_maps = _host_prep(inputs)
    res = run_bass_kernel_spmd(_cache['nc'], in_maps, list(range(N_CORES)))
    out = np.concatenate([res.results[c]['yout'] for c in range(N_CORES)])
    return out[:, None].astype(np.float32)

